# revision 17
# baseline (speedup 1.0000x reference)
"""Trainium2 Bass kernel for CIDER ISAB title encoder (v4).

x [2048, 32, 512] -> ISAB applied twice (shared params) -> mean over seq -> [2048, 512].
Data-parallel over 8 NeuronCores (256 batch elems each). bf16 matmul operands
(fp32 PSUM accumulate), fp32 softmax/LN statistics.

Key optimizations vs the original baseline:
- LN gamma/beta folded into consumer weights: every layernorm materializes only
  the normalized z=(x-mu)*rstd; W_o rows are pre-scaled by g0 (bias b0@Wo+bo via
  a rank-1 matmul into PSUM), and the mab-output ln1 affine is folded into the
  next layer's Wk/Wv/Wq (two weight variants for the X-vs-h1 input paths).
  All post-LN transposes then need a single plain PSUM->SBUF copy.
- rstd = Exp(-0.5*Ln(var+eps)); a get_activation_tables shim confines exp/ln to
  the one ACT table containing both, so there are no ACT_TABLE_LOADs.
- Residuals and free-axis biases accumulate in PSUM on the PE (identity-matmul
  / ones-matmul), with the full-tile write FIRST (has_written is per-region).
- mab1 scores packed 4-batches-per-matmul (garbage cols never read), softmax
  batched per 2-quad wave: 1 exp, 1 reduce, 1 reciprocal.
- Transposes write one 1-bank bf16 PSUM tile -> single merged ACT copy.
"""

import numpy as np

import concourse.bass as bass
import concourse.bacc as bacc
import concourse.mybir as mybir
from concourse.tile import TileContext
from concourse.masks import make_identity

F32 = mybir.dt.float32
BF16 = mybir.dt.bfloat16
AF = mybir.ActivationFunctionType
ALU = mybir.AluOpType
AX = mybir.AxisListType

D = 512
DT = 4          # d tiles of 128
H = 8           # heads
HP = 4          # head pairs
M = 16          # inducing points
S = 32          # seq len
NCORES = 8
NB = 2048 // NCORES     # 256 batches per core
G = 16                  # batches per group
EPS = 1e-5
SCALE = 1.0 / np.sqrt(np.float32(D))


def _ap(base, extra_dims, extra_off=0):
    """Manual AP: keep base partition dim, replace free dims."""
    return bass.AP(base.tensor, base.offset + extra_off, [base.ap[0]] + extra_dims)


STAGE = 99


def _patched_act_tables(orig_fn):
    """Wrap get_activation_tables so exp/ln are only offered by the one real
    table that contains BOTH (natural_log_exp_and_others). The table-load
    pass picks the first set containing a function; by default that puts
    exp in exp_and_others and ln in natural_log, so every rstd Ln->Exp pair
    reloads tables (1.3us each). Set ORDER and ids are unchanged, so the
    emitted act_func_set_id still names a real table whose true contents
    cover all our functions (identity/copy/exp/ln)."""
    def fn(arch):
        tabs = orig_fn(arch)
        keep = "natural_log_exp_and_others"
        if keep in tabs:
            import concourse.mybir as _mb
            drop = {_mb.ActivationFunctionType.Exp, _mb.ActivationFunctionType.Ln}
            tabs = {name: (s if name == keep else (set(s) - drop))
                    for name, s in tabs.items()}
        return tabs
    return fn


def build(nb=NB):
    ngroups = nb // G
    nc = bacc.Bacc(None, target_bir_lowering=False)

    x_d = nc.dram_tensor("x", [nb, S, D], F32, kind="ExternalInput")
    I_d = nc.dram_tensor("I", [1, M, D], F32, kind="ExternalInput")
    p_d = {}
    for mb in (0, 1):
        p_d[f"{mb}Wqkv"] = nc.dram_tensor(f"mab{mb}_Wqkv", [3, D, D], F32, kind="ExternalInput")
        p_d[f"{mb}bqkv"] = nc.dram_tensor(f"mab{mb}_bqkv", [3, D], F32, kind="ExternalInput")
        p_d[f"{mb}Wo"] = nc.dram_tensor(f"mab{mb}_Wo", [D, D], F32, kind="ExternalInput")
        p_d[f"{mb}bo"] = nc.dram_tensor(f"mab{mb}_bo", [D], F32, kind="ExternalInput")
        for nm in ("g0", "b0", "g1", "b1"):
            p_d[f"{mb}{nm}"] = nc.dram_tensor(f"mab{mb}_{nm}", [D], F32, kind="ExternalInput")
    out_d = nc.dram_tensor("out", [nb, D], F32, kind="ExternalOutput")

    with TileContext(nc) as tc:
        with tc.tile_pool(name="singles", bufs=1) as sg, \
             tc.tile_pool(name="work", bufs=1) as wk, \
             tc.tile_pool(name="small", bufs=3) as sm, \
             tc.tile_pool(name="ppA", bufs=3, space="PSUM") as ppA, \
             tc.tile_pool(name="ppB", bufs=2, space="PSUM") as ppB, \
             tc.tile_pool(name="ppC", bufs=1, space="PSUM") as ppC:

            # ============ SETUP ============
            id_f32 = sg.tile([128, 128], F32)
            make_identity(nc, id_f32)
            id_bf = sg.tile([128, 128], BF16)
            nc.vector.tensor_copy(id_bf, id_f32)

            eps_t = sg.tile([128, 1], F32)
            nc.vector.memset(eps_t, EPS)
            zrow = sg.tile([128, 1], F32)
            nc.vector.memset(zrow, 0.0)
            ones128 = sg.tile([128, 128], BF16)
            nc.vector.memset(ones128, 1.0 / 128.0)

            def zero_bf(dst_ap, nfree):
                nc.vector.tensor_copy(dst_ap, _ap(zrow[:, 0:1], [[0, nfree]]))

            # raw weights [128 (d_in part), DT (d_in tile), D (d_out)] bf16
            W = {}
            for mb in (0, 1):
                for qi, qn in enumerate(("q", "k", "v", "o")):
                    wr = sg.tile([128, DT, D], BF16, name=f"W{mb}{qn}")
                    src = p_d[f"{mb}Wqkv"][qi] if qn != "o" else p_d[f"{mb}Wo"]
                    stg = wk.tile([128, DT, D], F32, tag="xb", bufs=2, name=f"st{mb}{qn}")
                    nc.sync.dma_start(out=stg,
                                      in_=src.rearrange("(t p) d -> p t d", p=128))
                    nc.vector.tensor_copy(wr, stg)
                    W[f"{mb}{qn}"] = wr

            def pp_bias(src_ap, scale=None, name="b"):
                t = sg.tile([128, DT], F32, name=name)
                nc.sync.dma_start(out=t, in_=src_ap.rearrange("(t p) -> p t", p=128))
                if scale is not None:
                    nc.scalar.mul(t, t, float(scale))
                return t

            bk0_pp = pp_bias(p_d["0bqkv"][1], name="bk0")
            bq0s_pp = pp_bias(p_d["0bqkv"][0], SCALE, name="bq0s")
            bq1_pp = pp_bias(p_d["1bqkv"][0], name="bq1")
            bk1_pp = pp_bias(p_d["1bqkv"][1], name="bk1")
            g0_pp = {mb: pp_bias(p_d[f"{mb}g0"], name=f"g0pp{mb}") for mb in (0, 1)}
            g1_pp = {mb: pp_bias(p_d[f"{mb}g1"], name=f"g1pp{mb}") for mb in (0, 1)}
            b0c = {}
            b1c = {}
            for mb in (0, 1):
                t0 = pp_bias(p_d[f"{mb}b0"], name=f"b0pp{mb}")
                b0c[mb] = sg.tile([128, DT], BF16, name=f"b0c{mb}")
                nc.vector.tensor_copy(b0c[mb], t0)
                t1 = pp_bias(p_d[f"{mb}b1"], name=f"b1pp{mb}")
                b1c[mb] = sg.tile([128, DT], BF16, name=f"b1c{mb}")
                nc.vector.tensor_copy(b1c[mb], t1)
                if mb == 1:
                    b1_32 = t1              # for mean-pool fold
            g1s_pp = pp_bias(p_d["1g1"], 1.0 / S, name="g1spp")   # for mean-pool fold

            def bcast_row(row_f32, name):
                """[1,D] f32 sbuf row -> [128,D] bf16 broadcast tile."""
                rowb = sm.tile([1, D], BF16, tag="bcrowb", name=f"rb_{name}")
                nc.vector.tensor_copy(rowb, row_f32)
                t = sg.tile([128, D], BF16, name=f"bc_{name}")
                nc.gpsimd.partition_broadcast(t, rowb)
                return t

            def dma_row(src_ap, name):
                row = sm.tile([1, D], F32, tag="bcrow", name=f"r_{name}")
                nc.sync.dma_start(out=row, in_=src_ap[None, :])
                return row

            def bcast(src_ap, name):
                return bcast_row(dma_row(src_ap, name), name)

            ln_bc = {}
            for mb in (0, 1):
                for nm in ("g0", "b0"):
                    ln_bc[f"{mb}{nm}"] = bcast(p_d[f"{mb}{nm}"], f"ln{mb}{nm}")

            def bW_row(bcol_bf, Wt, name):
                """row = b @ W  (b feature-major col tile [128,DT] bf16) -> [1,D] f32."""
                ps = ppA.tile([1, D], F32, tag="lin")
                for k in range(DT):
                    nc.tensor.matmul(ps, bcol_bf[:, k:k + 1], Wt[:, k, :],
                                     start=(k == 0), stop=(k == DT - 1))
                row = sm.tile([1, D], F32, tag="bwrow", name=f"bw_{name}")
                nc.scalar.copy(row, ps)
                return row

            def row_to_pp(row_f32, name):
                """[1,D] f32 row -> feature-major [128,DT] f32 (PE transposes)."""
                t = sg.tile([128, DT], F32, name=f"pp_{name}")
                for k in range(DT):
                    ps = ppC.tile([128, 128], F32, tag="mp")
                    nc.tensor.transpose(ps[:, 0:1], row_f32[0:1, 128 * k:128 * (k + 1)],
                                        id_f32[0:1, 0:1])
                    nc.scalar.copy(t[:, k:k + 1], ps[:, 0:1])
                return t

            def scale_w_inplace(Wt, gpp):
                for k in range(DT):
                    nc.vector.tensor_scalar(Wt[:, k, :], Wt[:, k, :],
                                            gpp[:, k:k + 1], None, op0=ALU.mult)

            def scaled_w_copy(Wt, gpp, name):
                w2 = sg.tile([128, DT, D], BF16, name=name)
                for k in range(DT):
                    nc.vector.tensor_scalar(w2[:, k, :], Wt[:, k, :],
                                            gpp[:, k:k + 1], None, op0=ALU.mult)
                return w2

            # --- fc_o folds (both mabs): bo' = b0@Wo + bo ; Wo rows *= g0 ---
            bo_bc = {}
            for mb in (0, 1):
                r = bW_row(b0c[mb], W[f"{mb}o"], f"b0Wo{mb}")
                ro = dma_row(p_d[f"{mb}bo"], f"bo{mb}")
                nc.vector.tensor_add(r, r, ro)
                bo_bc[mb] = bcast_row(r, f"bo{mb}")
            # --- HT consumers (mab1 K/V always read Hm): fold mab0's g1/b1 ---
            r = bW_row(b1c[0], W["1k"], "b1W1k")
            bk1s_pp = row_to_pp(r, "b1W1k")
            nc.vector.tensor_add(bk1s_pp, bk1s_pp, bk1_pp)
            nc.scalar.mul(bk1s_pp, bk1s_pp, float(SCALE))
            r = bW_row(b1c[0], W["1v"], "b1W1v")
            rv = dma_row(p_d["1bqkv"][2], "bv1")
            nc.vector.tensor_add(r, r, rv)
            bv1_bc = bcast_row(r, "bv1")
            # --- h1 consumers (isab2's mab0 K/V + mab1 Q): fold mab1's g1/b1 ---
            r = bW_row(b1c[1], W["0k"], "b1W0k")
            bk0_2pp = row_to_pp(r, "b1W0k")
            nc.vector.tensor_add(bk0_2pp, bk0_2pp, bk0_pp)
            r_b1W0v = bW_row(b1c[1], W["0v"], "b1W0v")   # -> into Q0res_rep2
            r = bW_row(b1c[1], W["1q"], "b1W1q")
            bq1_2pp = row_to_pp(r, "b1W1q")
            nc.vector.tensor_add(bq1_2pp, bq1_2pp, bq1_pp)
            # --- scaled weight variants (after all bW rows are computed) ---
            g1s0_pp = pp_bias(p_d["0g1"], SCALE, name="g1s0")     # g1[0]*SCALE
            W0k_2 = scaled_w_copy(W["0k"], g1_pp[1], "W0k2")
            W0v_2 = scaled_w_copy(W["0v"], g1_pp[1], "W0v2")
            W1q_2 = scaled_w_copy(W["1q"], g1_pp[1], "W1q2")
            scale_w_inplace(W["1k"], g1s0_pp)    # W1k *= g1[0]*SCALE
            scale_w_inplace(W["1v"], g1_pp[0])   # W1v *= g1[0]
            scale_w_inplace(W["0o"], g0_pp[0])   # Wo  *= g0
            scale_w_inplace(W["1o"], g0_pp[1])

            # I -> IT [128, DT, M] bf16 (feature-major inducing points)
            Ibf = sm.tile([M, D], F32, tag="bcrow", name="Ibf")
            nc.sync.dma_start(out=Ibf, in_=I_d[0])
            Ib = sg.tile([M, D], BF16, name="Ib")
            nc.vector.tensor_copy(Ib, Ibf)
            IT = sg.tile([128, DT, M], BF16)
            for m in range(DT):
                ps = ppB.tile([128, DT, 128], BF16, tag="tp")
                nc.tensor.transpose(ps[:, 0, 0:M], Ib[:, 128 * m:128 * (m + 1)],
                                    id_bf[0:M, 0:M])
                nc.scalar.copy(IT[:, m, :], ps[:, 0, 0:M])

            # Q0T = (I @ Wq0 + bq0) * SCALE, feature-major
            Q0T = sg.tile([128, DT, M], BF16)
            for m in range(DT):
                ps = ppA.tile([128, M], F32, tag="lin")
                for k in range(DT):
                    nc.tensor.matmul(ps, W["0q"][:, k, 128 * m:128 * (m + 1)],
                                     IT[:, k, :], start=(k == 0), stop=(k == DT - 1))
                nc.scalar.activation(Q0T[:, m, :], ps, AF.Identity,
                                     bias=bq0s_pp[:, m:m + 1], scale=float(SCALE))

            # Q0blk [128, HP, 2M] block-diag (head pair) for scores0
            Q0blk = sg.tile([128, HP, 2 * M], BF16)
            zero_bf(Q0blk, HP * 2 * M)
            for hp in range(HP):
                nc.vector.tensor_copy(Q0blk[0:64, hp, 0:M], Q0T[0:64, hp, :])
                nc.vector.tensor_copy(Q0blk[64:128, hp, M:2 * M], Q0T[64:128, hp, :])

            # Q0res_rep [128, D] bf16: 8x-replicated (I@Wq0 + bq0 + bv0), token-major
            r0 = sm.tile([1, D], F32, tag="bcrow")
            nc.sync.dma_start(out=r0, in_=p_d["0bqkv"][0][None, :])
            r1 = sm.tile([1, D], F32, tag="bcrow")
            nc.sync.dma_start(out=r1, in_=p_d["0bqkv"][2][None, :])
            nc.vector.tensor_add(r0, r0, r1)
            bqv0_bc = sg.tile([128, D], F32)
            nc.gpsimd.partition_broadcast(bqv0_bc, r0)
            ITrep = sg.tile([128, DT, 128], BF16)
            for k in range(DT):
                nc.vector.tensor_copy(ITrep[:, k, :],
                                      _ap(IT[:, k, :], [[0, 8], [1, M]]))
            psq = ppA.tile([128, D], F32, tag="lin")
            for k in range(DT):
                nc.tensor.matmul(psq, ITrep[:, k, :], W["0q"][:, k, :],
                                 start=(k == 0), stop=(k == DT - 1))
            Q0res_rep = sg.tile([128, D], BF16)
            nc.vector.tensor_add(Q0res_rep, psq, bqv0_bc)
            # isab2 variant adds the folded b1[1]@W0v row (A rows sum to 1)
            Q0res_rep2 = sg.tile([128, D], BF16)
            nc.vector.tensor_add(Q0res_rep2, Q0res_rep, bcast_row(r_b1W0v, "b1W0v"))

            # persistent zero-padded block-diag A^T stores (off-diag stays 0)
            AQ0 = sg.tile([128, 4, HP, 2, 4, M], BF16)  # [part(4g x 32k), j, hp, i, gcol, q]
            zero_bf(AQ0, 4 * HP * 2 * 4 * M)
            AQ1 = sg.tile([128, H, 2, 4, S], BF16)      # [part(8b x 16k), h, half, bcol, q]
            zero_bf(AQ1, H * 2 * 4 * S)
            A1sb = sg.tile([128, H, 2, M], BF16)        # [q-rows, h, batch-parity, k]
            zero_bf(A1sb, H * 2 * M)
            # K1T variants with one head-parity's rows zeroed (so scores1 can
            # contract K=128 from row 0 and avoid the broken (64,96) PE quadrant)
            K1Te = sg.tile([128, DT, 256], BF16)
            zero_bf(K1Te, DT * 256)
            K1To = sg.tile([128, DT, 256], BF16)
            zero_bf(K1To, DT * 256)

            # ============ helpers ============
            def linear_fm(Wsb, inT, toks, bias_pp, tag):
                outT = wk.tile([128, DT, 512], BF16, tag=tag, bufs=2, name=f"fm_{tag}")
                for m in range(DT):
                    ps = ppA.tile([128, toks], F32, tag="lin")
                    for k in range(DT):
                        nc.tensor.matmul(ps, Wsb[:, k, 128 * m:128 * (m + 1)],
                                         inT[:, k, :toks], start=(k == 0), stop=(k == DT - 1))
                    if bias_pp is not None:
                        nc.scalar.activation(outT[:, m, :toks], ps, AF.Identity,
                                             bias=bias_pp[:, m:m + 1])
                    else:
                        nc.scalar.copy(outT[:, m, :toks], ps)
                return outT

            def ln_z(x_sb, ztag):
                """z = (x - mu) * rstd, bf16. rstd via Ln+Exp (one ACT table)."""
                st = sm.tile([128, 6], F32, tag="lnst")
                nc.vector.bn_stats(st, x_sb)
                mv = sm.tile([128, 2], F32, tag="lnmv")
                nc.vector.bn_aggr(mv, st)
                lnv = sm.tile([128, 1], F32, tag=f"{ztag}l")
                nc.scalar.activation(lnv, mv[:, 1:2], AF.Ln, bias=eps_t[:, 0:1])
                rstd = sm.tile([128, 1], F32, tag=f"{ztag}r")
                nc.scalar.activation(rstd, lnv, AF.Exp, scale=-0.5)
                z = sm.tile([128, D], BF16, tag=ztag, name=ztag)
                nc.vector.tensor_scalar(z, x_sb, mv[:, 0:1], rstd[:, 0:1],
                                        op0=ALU.subtract, op1=ALU.mult)
                return z

            def transpose_fm(isl, dst_ap):
                """PE-transpose island [128 toks, 512] bf16 -> feature-major via
                one 1-bank PSUM tile and a single merged copy."""
                ps = ppB.tile([128, DT, 128], BF16, tag="tp")
                for m in range(DT):
                    nc.tensor.transpose(ps[:, m, :], isl[:, 128 * m:128 * (m + 1)], id_bf)
                nc.scalar.copy(dst_ap, ps)

            # ============ one ISAB for one group of 16 batches ============
            def dummy_out(g):
                osb = sm.tile([G, D], F32, tag="osb", bufs=2)
                nc.vector.memset(osb, 0.0)
                nc.sync.dma_start(out=out_d[G * g:G * (g + 1), :], in_=osb)

            def isab(inT, g, last):
                # weight/bias variants: isab2 consumes h1 = ln1-normalized only,
                # with mab1's g1/b1 folded into these
                Wk0 = W["0k"] if not last else W0k_2
                bk0v = bk0_pp if not last else bk0_2pp
                Wv0 = W["0v"] if not last else W0v_2
                Wq1 = W["1q"] if not last else W1q_2
                bq1v = bq1_pp if not last else bq1_2pp
                Q0res_v = Q0res_rep if not last else Q0res_rep2

                # ---- mab0: Hm = MAB(I, X) ----
                if STAGE < 2:
                    return None
                KT = linear_fm(Wk0, inT, 512, bk0v, tag="kt")
                V0t = wk.tile([128, 4, D], BF16, tag="v0t", bufs=2, name="v0t")
                for i in range(4):
                    ps = ppA.tile([128, D], F32, tag="lin")
                    for k in range(DT):
                        nc.tensor.matmul(ps, inT[:, k, 128 * i:128 * (i + 1)],
                                         Wv0[:, k, :], start=(k == 0), stop=(k == DT - 1))
                    nc.scalar.copy(V0t[:, i, :], ps)

                ps_s = ppA.tile([128, 512], F32, tag="lin")
                for hp in range(HP):
                    nc.tensor.matmul(ps_s[32 * hp:32 * (hp + 1), :], Q0blk[:, hp, :],
                                     KT[:, hp, :], start=True, stop=True,
                                     tile_position=(0, 32 * hp))
                E0 = sm.tile([128, 512], F32, tag="e0")
                nc.scalar.activation(E0, ps_s, AF.Exp)
                den = sm.tile([128, G], F32, tag="den0")
                nc.vector.tensor_reduce(den, E0.rearrange("p (b k) -> p b k", k=S),
                                        axis=AX.X, op=ALU.add)
                nc.vector.reciprocal(den, den)
                A0 = sm.tile([128, 512], BF16, tag="a0")
                nc.vector.tensor_tensor(
                    A0.rearrange("p (b k) -> p b k", k=S), E0.rearrange("p (b k) -> p b k", k=S),
                    _ap(den[:, :], [[1, G], [0, S]]), op=ALU.mult)
                # A^T for all 4 quads into the block-diag store
                A0v = A0.rearrange("p (j q k) -> p j q k", q=4, k=S)
                for hp in range(HP):
                    for gq in range(4):
                        nc.vector.transpose(
                            _ap(AQ0[32 * gq:32 * (gq + 1), 0, hp, 0, gq, :],
                                [[512, 4], [64, 2], [1, M]]),
                            A0v[32 * hp:32 * (hp + 1), :, gq, :])
                if STAGE < 3:
                    return None
                isl0 = []
                for jj in range(2):          # two islands of 128 tokens (8 batches)
                    ps_av = ppA.tile([128, D], F32, tag="lin")
                    # full-tile residual (I@Wq0+bq0+bv0) FIRST with start=True;
                    # AV matmuls then accumulate onto it (order-robust)
                    nc.tensor.matmul(ps_av, id_bf, Q0res_v,
                                     start=True, stop=False, skip_group_check=True)
                    for j2 in range(2):
                        j = 2 * jj + j2
                        for hp in range(HP):
                            for i in range(2):
                                h = 2 * hp + i
                                nc.tensor.matmul(
                                    ps_av[64 * j2:64 * j2 + 64, 64 * h:64 * (h + 1)],
                                    AQ0[:, j, hp, i, :, :].rearrange("p g q -> p (g q)"),
                                    V0t[:, j, 64 * h:64 * (h + 1)],
                                    start=False,
                                    stop=(j2 == 1 and hp == HP - 1 and i == 1),
                                    tile_position=(0, 64 * j2), skip_group_check=True)
                    O0 = sm.tile([128, D], BF16, tag="o0")
                    nc.scalar.copy(O0, ps_av)
                    z0 = ln_z(O0, "z0")
                    # L0 = g0*z0 + b0 (token-major residual for the fc block)
                    t0 = sm.tile([128, D], BF16, tag="lnt1")
                    nc.vector.tensor_tensor(t0, z0, ln_bc["0g0"], op=ALU.mult)
                    L0 = sm.tile([128, D], BF16, tag="ln0out")
                    nc.vector.tensor_tensor(L0, t0, ln_bc["0b0"], op=ALU.add)
                    OT = wk.tile([128, DT, 128], BF16, tag="ot", bufs=2)
                    transpose_fm(z0, OT[:, :, :])
                    psf = ppA.tile([128, D], F32, tag="lin")
                    nc.tensor.matmul(psf, ones128, bo_bc[0],
                                     start=True, stop=False, skip_group_check=True)
                    for k in range(DT):
                        nc.tensor.matmul(psf, OT[:, k, :], W["0o"][:, k, :],
                                         start=False, stop=(k == DT - 1),
                                         skip_group_check=True)
                    O2 = sm.tile([128, D], BF16, tag="o2")
                    nc.vector.scalar_tensor_tensor(O2, psf, 0.0, L0,
                                                   op0=ALU.max, op1=ALU.add)
                    isl0.append(ln_z(O2, "lnN"))
                HT = wk.tile([128, DT, 272], BF16, tag="ht", bufs=2)
                for t2 in range(2):
                    transpose_fm(isl0[t2], HT[:, :, 128 * t2:128 * t2 + 128])

                # ---- mab1: out = MAB(X, Hm) ----
                if STAGE < 4:
                    return None
                Q1Tb = wk.tile([128, DT, 512], BF16, tag="q1b", bufs=2, name="q1b")
                for m in range(DT):
                    ps = ppA.tile([128, 512], F32, tag="lin")
                    for k in range(DT):
                        nc.tensor.matmul(ps, Wq1[:, k, 128 * m:128 * (m + 1)],
                                         inT[:, k, :], start=(k == 0), stop=(k == DT - 1))
                    nc.scalar.activation(Q1Tb[:, m, :], ps, AF.Identity,
                                         bias=bq1v[:, m:m + 1])
                for m in range(DT):
                    ps = ppA.tile([128, 256], F32, tag="lin")
                    for k in range(DT):
                        nc.tensor.matmul(ps, W["1k"][:, k, 128 * m:128 * (m + 1)],
                                         HT[:, k, :256], start=(k == 0), stop=(k == DT - 1))
                    nc.scalar.activation(K1Te[0:64, m, :], ps[0:64, :], AF.Identity,
                                         bias=bk1s_pp[0:64, m:m + 1])
                    nc.scalar.activation(K1To[64:128, m, :], ps[64:128, :], AF.Identity,
                                         bias=bk1s_pp[64:128, m:m + 1])
                V1t = wk.tile([128, 2, D], BF16, tag="v1t", bufs=2, name="v1t")
                for i in range(2):
                    ps = ppA.tile([128, D], F32, tag="lin")
                    for k in range(DT):
                        nc.tensor.matmul(ps, HT[:, k, 128 * i:128 * (i + 1)],
                                         W["1v"][:, k, :], start=(k == 0),
                                         stop=(k == DT - 1))
                    nc.scalar.copy(V1t[:, i, :], ps)

                if STAGE < 5:
                    return None
                if not last:
                    h1T = wk.tile([128, DT, 512], BF16, tag="fmX", bufs=2, name="h1T")
                else:
                    macc = sm.tile([128, DT, G], F32, tag="macc")
                # ---- scores in 2 waves of 2 quads; one MM per (quad, head)
                # covering all 4 batches (3/4 garbage cols, never read) ----
                for w in range(2):
                  ps_w = ppB.tile([128, 2, H, 4, M], F32, tag="s1", bufs=1)
                  for j2 in range(2):
                    j = 2 * w + j2
                    for h in range(H):
                        hp, i = h // 2, h % 2
                        K1v = K1Te if i == 0 else K1To
                        nc.tensor.matmul(
                            ps_w[:, j2, h, :, :].rearrange("p b k -> p (b k)"),
                            Q1Tb[:, hp, 128 * j:128 * (j + 1)],
                            K1v[:, hp, M * 4 * j:M * 4 * (j + 1)],
                            start=True, stop=True)
                  E1w = sm.tile([128, 2, H, 4, M], F32, tag="e1")
                  nc.scalar.activation(E1w.rearrange("p a h b k -> p (a h b k)"),
                                       ps_w.rearrange("p a h b k -> p (a h b k)"),
                                       AF.Exp)
                  den1 = sm.tile([128, 2, H, 4], F32, tag="den1")
                  nc.vector.tensor_reduce(den1, E1w, axis=AX.X, op=ALU.add)
                  nc.vector.reciprocal(den1.rearrange("p a h b -> p (a h b)"),
                                       den1.rearrange("p a h b -> p (a h b)"))
                  for j2 in range(2):
                    j = 2 * w + j2
                    j8, half = j // 2, j % 2
                    # normalize valid diagonal blocks into A1sb (pad stays zero)
                    for gq in range(4):
                        sl = slice(32 * gq, 32 * (gq + 1))
                        nc.vector.tensor_tensor(
                            A1sb[sl, :, gq % 2, :], E1w[sl, j2, :, gq, :],
                            _ap(den1[sl, j2, :, gq], [[4, H], [0, M]]), op=ALU.mult)
                    # A^T blocks: [32 q, (b2,k)] -> [32 (b2,k), q] at row 32*(2*half+gq//2)
                    for gq in range(4):
                        prow = 32 * (2 * half + gq // 2)
                        nc.vector.transpose(
                            _ap(AQ1[prow:prow + 32, 0, half, gq, :],
                                [[2 * 4 * S, H], [1, S]]),
                            A1sb[32 * gq:32 * (gq + 1), :, :, :])
                    if STAGE == 43:
                        continue
                    ps_av1 = ppA.tile([128, D], F32, tag="lin")
                    # bv1 broadcast FIRST covering the full tile (A rows sum to 1,
                    # so A@(V+bv1) == A@V + bv1), then residual Q1^T transposes and
                    # AV matmuls accumulate onto it
                    nc.tensor.matmul(ps_av1, ones128, bv1_bc,
                                     start=True, stop=False, skip_group_check=True)
                    for m in range(DT):
                        nc.tensor.matmul(ps_av1[:, 128 * m:128 * (m + 1)],
                                         Q1Tb[:, m, 128 * j:128 * (j + 1)], id_bf,
                                         start=False, stop=False,
                                         skip_group_check=True)
                    for h in range(H):
                        nc.tensor.matmul(ps_av1[:, 64 * h:64 * (h + 1)],
                                         AQ1[:, h, half, :, :].rearrange("p g q -> p (g q)"),
                                         V1t[:, j8, 64 * h:64 * (h + 1)],
                                         start=False, stop=(h == H - 1),
                                         skip_group_check=True)
                    if STAGE == 44:
                        continue
                    O1 = sm.tile([128, D], BF16, tag="o0")
                    nc.scalar.copy(O1, ps_av1)
                    z1 = ln_z(O1, "z0")
                    t1 = sm.tile([128, D], BF16, tag="lnt1")
                    nc.vector.tensor_tensor(t1, z1, ln_bc["1g0"], op=ALU.mult)
                    L1 = sm.tile([128, D], BF16, tag="ln0out")
                    nc.vector.tensor_tensor(L1, t1, ln_bc["1b0"], op=ALU.add)
                    OT1 = wk.tile([128, DT, 128], BF16, tag="ot", bufs=2)
                    transpose_fm(z1, OT1[:, :, :])
                    psf = ppA.tile([128, D], F32, tag="lin")
                    nc.tensor.matmul(psf, ones128, bo_bc[1],
                                     start=True, stop=False, skip_group_check=True)
                    for k in range(DT):
                        nc.tensor.matmul(psf, OT1[:, k, :], W["1o"][:, k, :],
                                         start=False, stop=(k == DT - 1),
                                         skip_group_check=True)
                    O2 = sm.tile([128, D], BF16, tag="o2")
                    nc.vector.scalar_tensor_tensor(O2, psf, 0.0, L1,
                                                   op0=ALU.max, op1=ALU.add)
                    OUTj = ln_z(O2, "lnN")
                    if not last:
                        transpose_fm(OUTj, h1T[:, :, 128 * j:128 * j + 128])
                    else:
                        ps = ppB.tile([128, DT, 128], BF16, tag="tp")
                        for m in range(DT):
                            nc.tensor.transpose(ps[:, m, :], OUTj[:, 128 * m:128 * (m + 1)], id_bf)
                        nc.vector.tensor_reduce(
                            macc[:, :, 4 * j:4 * (j + 1)],
                            ps.rearrange("p m (b s) -> p m b s", s=S),
                            axis=AX.X, op=ALU.add)
                if STAGE < 46:
                    return None
                if not last:
                    return h1T
                # pooled = g1/S * sum + b1, applied feature-major, then transpose out
                macc2 = sm.tile([128, DT, G], F32, tag="macc2")
                for m in range(DT):
                    nc.scalar.activation(macc2[:, m, :], macc[:, m, :], AF.Identity,
                                         bias=b1_32[:, m:m + 1], scale=g1s_pp[:, m:m + 1])
                osb = sm.tile([G, D], F32, tag="osb", bufs=2)
                for m in range(DT):
                    ps = ppC.tile([128, 128], F32, tag="mp")
                    nc.tensor.transpose(ps[0:G, :], macc2[:, m, :], id_f32)
                    nc.scalar.copy(osb[:, 128 * m:128 * (m + 1)], ps[0:G, :])
                nc.sync.dma_start(out=out_d[G * g:G * (g + 1), :], in_=osb)
                return None

            # ============ main loop ============
            x_flat = x_d.rearrange("b s d -> (b s) d")
            for g in range(ngroups):
                Xb = wk.tile([128, 4, D], F32, tag="xb", bufs=2)
                for i in range(4):
                    nc.sync.dma_start(
                        out=Xb[:, i, :],
                        in_=x_flat[512 * g + 128 * i: 512 * g + 128 * (i + 1), :])
                Xbb = wk.tile([128, 4, D], BF16, tag="xbb", bufs=2)
                nc.vector.tensor_copy(Xbb, Xb)
                XT = wk.tile([128, DT, 512], BF16, tag="fmX", bufs=2, name="XT")
                for i in range(4):
                    ps = ppB.tile([128, DT, 128], BF16, tag="tp")
                    for m in range(DT):
                        nc.tensor.transpose(ps[:, m, :], Xbb[:, i, 128 * m:128 * (m + 1)], id_bf)
                    nc.scalar.copy(XT[:, :, 128 * i:128 * (i + 1)], ps)
                h1T = isab(XT, g, last=False)
                if h1T is None or STAGE < 6:
                    dummy_out(g)
                    continue
                isab(h1T, g, last=True)

    _orig = bacc.get_activation_tables
    bacc.get_activation_tables = _patched_act_tables(_orig)
    try:
        nc.finalize()
    finally:
        bacc.get_activation_tables = _orig
    return nc


_CACHE = {}


def _get_nc(nb):
    if nb not in _CACHE:
        _CACHE[nb] = build(nb)
    return _CACHE[nb]


def kernel(**inputs):
    from concourse.bass_utils import run_bass_kernel_spmd

    x = np.ascontiguousarray(inputs["x"], dtype=np.float32)
    nbatch = x.shape[0]
    per = nbatch // NCORES
    nc = _get_nc(per)
    shared = {k: np.ascontiguousarray(np.asarray(v), dtype=np.float32)
              for k, v in inputs.items() if k != "x"}
    in_maps = [dict(shared, x=x[c * per:(c + 1) * per]) for c in range(NCORES)]
    res = run_bass_kernel_spmd(nc, in_maps, core_ids=list(range(NCORES)))
    return np.concatenate([r["out"] for r in res.results], axis=0)


# revision 19
# speedup vs baseline: 1.1983x; 1.1983x over previous
"""Trainium2 Bass kernel for CIDER ISAB title encoder (v4).

x [2048, 32, 512] -> ISAB applied twice (shared params) -> mean over seq -> [2048, 512].
Data-parallel over 8 NeuronCores (256 batch elems each). bf16 matmul operands
(fp32 PSUM accumulate), fp32 softmax/LN statistics.

Key optimizations vs the original baseline:
- LN gamma/beta folded into consumer weights: every layernorm materializes only
  the normalized z=(x-mu)*rstd; W_o rows are pre-scaled by g0 (bias b0@Wo+bo via
  a rank-1 matmul into PSUM), and the mab-output ln1 affine is folded into the
  next layer's Wk/Wv/Wq (two weight variants for the X-vs-h1 input paths).
  All post-LN transposes then need a single plain PSUM->SBUF copy.
- rstd = Exp(-0.5*Ln(var+eps)); a get_activation_tables shim confines exp/ln to
  the one ACT table containing both, so there are no ACT_TABLE_LOADs.
- Residuals and free-axis biases accumulate in PSUM on the PE (identity-matmul
  / ones-matmul), with the full-tile write FIRST (has_written is per-region).
- mab1 scores packed 4-batches-per-matmul (garbage cols never read), softmax
  batched per 2-quad wave: 1 exp, 1 reduce, 1 reciprocal.
- Transposes write one 1-bank bf16 PSUM tile -> single merged ACT copy.
"""

import numpy as np

import concourse.bass as bass
import concourse.bacc as bacc
import concourse.mybir as mybir
from concourse.tile import TileContext
from concourse.masks import make_identity

F32 = mybir.dt.float32
BF16 = mybir.dt.bfloat16
AF = mybir.ActivationFunctionType
ALU = mybir.AluOpType
AX = mybir.AxisListType

D = 512
DT = 4          # d tiles of 128
H = 8           # heads
HP = 4          # head pairs
M = 16          # inducing points
S = 32          # seq len
NCORES = 8
NB = 2048 // NCORES     # 256 batches per core
G = 16                  # batches per group
EPS = 1e-5
SCALE = 1.0 / np.sqrt(np.float32(D))


def _ap(base, extra_dims, extra_off=0):
    """Manual AP: keep base partition dim, replace free dims."""
    return bass.AP(base.tensor, base.offset + extra_off, [base.ap[0]] + extra_dims)


STAGE = 99


def _patched_act_tables(orig_fn):
    """Wrap get_activation_tables so exp/ln are only offered by the one real
    table that contains BOTH (natural_log_exp_and_others). The table-load
    pass picks the first set containing a function; by default that puts
    exp in exp_and_others and ln in natural_log, so every rstd Ln->Exp pair
    reloads tables (1.3us each). Set ORDER and ids are unchanged, so the
    emitted act_func_set_id still names a real table whose true contents
    cover all our functions (identity/copy/exp/ln)."""
    def fn(arch):
        tabs = orig_fn(arch)
        keep = "natural_log_exp_and_others"
        if keep in tabs:
            import concourse.mybir as _mb
            drop = {_mb.ActivationFunctionType.Exp, _mb.ActivationFunctionType.Ln}
            tabs = {name: (s if name == keep else (set(s) - drop))
                    for name, s in tabs.items()}
        return tabs
    return fn


def build(nb=NB):
    ngroups = nb // G
    nc = bacc.Bacc(None, target_bir_lowering=False)

    x_d = nc.dram_tensor("x", [nb, S, D], F32, kind="ExternalInput")
    I_d = nc.dram_tensor("I", [1, M, D], F32, kind="ExternalInput")
    p_d = {}
    for mb in (0, 1):
        p_d[f"{mb}Wqkv"] = nc.dram_tensor(f"mab{mb}_Wqkv", [3, D, D], F32, kind="ExternalInput")
        p_d[f"{mb}bqkv"] = nc.dram_tensor(f"mab{mb}_bqkv", [3, D], F32, kind="ExternalInput")
        p_d[f"{mb}Wo"] = nc.dram_tensor(f"mab{mb}_Wo", [D, D], F32, kind="ExternalInput")
        p_d[f"{mb}bo"] = nc.dram_tensor(f"mab{mb}_bo", [D], F32, kind="ExternalInput")
        for nm in ("g0", "b0", "g1", "b1"):
            p_d[f"{mb}{nm}"] = nc.dram_tensor(f"mab{mb}_{nm}", [D], F32, kind="ExternalInput")
    out_d = nc.dram_tensor("out", [nb, D], F32, kind="ExternalOutput")

    with TileContext(nc) as tc:
        with tc.tile_pool(name="singles", bufs=1) as sg, \
             tc.tile_pool(name="work", bufs=1) as wk, \
             tc.tile_pool(name="small", bufs=2) as sm, \
             tc.tile_pool(name="ppA", bufs=3, space="PSUM") as ppA, \
             tc.tile_pool(name="ppB", bufs=2, space="PSUM") as ppB, \
             tc.tile_pool(name="ppC", bufs=1, space="PSUM") as ppC:

            # ============ SETUP ============
            id_f32 = sg.tile([128, 128], F32)
            make_identity(nc, id_f32)
            id_bf = sg.tile([128, 128], BF16)
            nc.vector.tensor_copy(id_bf, id_f32)

            eps_t = sg.tile([128, 1], F32)
            nc.vector.memset(eps_t, EPS)
            zrow = sg.tile([128, 1], F32)
            nc.vector.memset(zrow, 0.0)
            ones128 = sg.tile([128, 128], BF16)
            nc.vector.memset(ones128, 1.0 / 128.0)

            def zero_bf(dst_ap, nfree):
                nc.vector.tensor_copy(dst_ap, _ap(zrow[:, 0:1], [[0, nfree]]))

            # raw weights [128 (d_in part), DT (d_in tile), D (d_out)] bf16
            W = {}
            for mb in (0, 1):
                for qi, qn in enumerate(("q", "k", "v", "o")):
                    wr = sg.tile([128, DT, D], BF16, name=f"W{mb}{qn}")
                    src = p_d[f"{mb}Wqkv"][qi] if qn != "o" else p_d[f"{mb}Wo"]
                    stg = wk.tile([128, DT, D], F32, tag="xb", bufs=2, name=f"st{mb}{qn}")
                    nc.sync.dma_start(out=stg,
                                      in_=src.rearrange("(t p) d -> p t d", p=128))
                    nc.vector.tensor_copy(wr, stg)
                    W[f"{mb}{qn}"] = wr

            def pp_bias(src_ap, scale=None, name="b"):
                t = sg.tile([128, DT], F32, name=name)
                nc.sync.dma_start(out=t, in_=src_ap.rearrange("(t p) -> p t", p=128))
                if scale is not None:
                    nc.scalar.mul(t, t, float(scale))
                return t

            bk0_pp = pp_bias(p_d["0bqkv"][1], name="bk0")
            bq0s_pp = pp_bias(p_d["0bqkv"][0], SCALE, name="bq0s")
            bq1_pp = pp_bias(p_d["1bqkv"][0], name="bq1")
            bk1_pp = pp_bias(p_d["1bqkv"][1], name="bk1")
            g0_pp = {mb: pp_bias(p_d[f"{mb}g0"], name=f"g0pp{mb}") for mb in (0, 1)}
            g1_pp = {mb: pp_bias(p_d[f"{mb}g1"], name=f"g1pp{mb}") for mb in (0, 1)}
            b0c = {}
            b1c = {}
            for mb in (0, 1):
                t0 = pp_bias(p_d[f"{mb}b0"], name=f"b0pp{mb}")
                b0c[mb] = sg.tile([128, DT], BF16, name=f"b0c{mb}")
                nc.vector.tensor_copy(b0c[mb], t0)
                t1 = pp_bias(p_d[f"{mb}b1"], name=f"b1pp{mb}")
                b1c[mb] = sg.tile([128, DT], BF16, name=f"b1c{mb}")
                nc.vector.tensor_copy(b1c[mb], t1)
                if mb == 1:
                    b1_32 = t1              # for mean-pool fold
            g1s_pp = pp_bias(p_d["1g1"], 1.0 / S, name="g1spp")   # for mean-pool fold

            def bcast_row(row_f32, name):
                """[1,D] f32 sbuf row -> [128,D] bf16 broadcast tile."""
                rowb = sm.tile([1, D], BF16, tag="bcrowb", name=f"rb_{name}")
                nc.vector.tensor_copy(rowb, row_f32)
                t = sg.tile([128, D], BF16, name=f"bc_{name}")
                nc.gpsimd.partition_broadcast(t, rowb)
                return t

            def dma_row(src_ap, name):
                row = sm.tile([1, D], F32, tag="bcrow", name=f"r_{name}")
                nc.sync.dma_start(out=row, in_=src_ap[None, :])
                return row

            def bcast(src_ap, name):
                return bcast_row(dma_row(src_ap, name), name)

            ln_bc = {}
            for mb in (0, 1):
                for nm in ("g0", "b0"):
                    ln_bc[f"{mb}{nm}"] = bcast(p_d[f"{mb}{nm}"], f"ln{mb}{nm}")

            def bW_row(bcol_bf, Wt, name):
                """row = b @ W  (b feature-major col tile [128,DT] bf16) -> [1,D] f32."""
                ps = ppA.tile([1, D], F32, tag="lin")
                for k in range(DT):
                    nc.tensor.matmul(ps, bcol_bf[:, k:k + 1], Wt[:, k, :],
                                     start=(k == 0), stop=(k == DT - 1))
                row = sm.tile([1, D], F32, tag="bwrow", name=f"bw_{name}")
                nc.scalar.copy(row, ps)
                return row

            def row_to_pp(row_f32, name):
                """[1,D] f32 row -> feature-major [128,DT] f32 (PE transposes)."""
                t = sg.tile([128, DT], F32, name=f"pp_{name}")
                for k in range(DT):
                    ps = ppC.tile([128, 128], F32, tag="mp")
                    nc.tensor.transpose(ps[:, 0:1], row_f32[0:1, 128 * k:128 * (k + 1)],
                                        id_f32[0:1, 0:1])
                    nc.scalar.copy(t[:, k:k + 1], ps[:, 0:1])
                return t

            def scale_w_inplace(Wt, gpp):
                for k in range(DT):
                    nc.vector.tensor_scalar(Wt[:, k, :], Wt[:, k, :],
                                            gpp[:, k:k + 1], None, op0=ALU.mult)

            def scaled_w_copy(Wt, gpp, name):
                w2 = sg.tile([128, DT, D], BF16, name=name)
                for k in range(DT):
                    nc.vector.tensor_scalar(w2[:, k, :], Wt[:, k, :],
                                            gpp[:, k:k + 1], None, op0=ALU.mult)
                return w2

            # --- fc_o folds (both mabs): bo' = b0@Wo + bo ; Wo rows *= g0 ---
            bo_bc = {}
            for mb in (0, 1):
                r = bW_row(b0c[mb], W[f"{mb}o"], f"b0Wo{mb}")
                ro = dma_row(p_d[f"{mb}bo"], f"bo{mb}")
                nc.vector.tensor_add(r, r, ro)
                bo_bc[mb] = bcast_row(r, f"bo{mb}")
            # --- HT consumers (mab1 K/V always read Hm): fold mab0's g1/b1 ---
            r = bW_row(b1c[0], W["1k"], "b1W1k")
            bk1s_pp = row_to_pp(r, "b1W1k")
            nc.vector.tensor_add(bk1s_pp, bk1s_pp, bk1_pp)
            nc.scalar.mul(bk1s_pp, bk1s_pp, float(SCALE))
            r = bW_row(b1c[0], W["1v"], "b1W1v")
            rv = dma_row(p_d["1bqkv"][2], "bv1")
            nc.vector.tensor_add(r, r, rv)
            bv1_bc = bcast_row(r, "bv1")
            # --- h1 consumers (isab2's mab0 K/V + mab1 Q): fold mab1's g1/b1 ---
            r = bW_row(b1c[1], W["0k"], "b1W0k")
            bk0_2pp = row_to_pp(r, "b1W0k")
            nc.vector.tensor_add(bk0_2pp, bk0_2pp, bk0_pp)
            r_b1W0v = bW_row(b1c[1], W["0v"], "b1W0v")   # -> into Q0res_rep2
            r = bW_row(b1c[1], W["1q"], "b1W1q")
            bq1_2pp = row_to_pp(r, "b1W1q")
            nc.vector.tensor_add(bq1_2pp, bq1_2pp, bq1_pp)
            # --- scaled weight variants (after all bW rows are computed) ---
            g1s0_pp = pp_bias(p_d["0g1"], SCALE, name="g1s0")     # g1[0]*SCALE
            W0k_2 = scaled_w_copy(W["0k"], g1_pp[1], "W0k2")
            W0v_2 = scaled_w_copy(W["0v"], g1_pp[1], "W0v2")
            W1q_2 = scaled_w_copy(W["1q"], g1_pp[1], "W1q2")
            scale_w_inplace(W["1k"], g1s0_pp)    # W1k *= g1[0]*SCALE
            scale_w_inplace(W["1v"], g1_pp[0])   # W1v *= g1[0]
            scale_w_inplace(W["0o"], g0_pp[0])   # Wo  *= g0
            scale_w_inplace(W["1o"], g0_pp[1])

            # I -> IT [128, DT, M] bf16 (feature-major inducing points)
            Ibf = sm.tile([M, D], F32, tag="bcrow", name="Ibf")
            nc.sync.dma_start(out=Ibf, in_=I_d[0])
            Ib = sg.tile([M, D], BF16, name="Ib")
            nc.vector.tensor_copy(Ib, Ibf)
            IT = sg.tile([128, DT, M], BF16)
            for m in range(DT):
                ps = ppB.tile([128, DT, 128], BF16, tag="tp")
                nc.tensor.transpose(ps[:, 0, 0:M], Ib[:, 128 * m:128 * (m + 1)],
                                    id_bf[0:M, 0:M])
                nc.scalar.copy(IT[:, m, :], ps[:, 0, 0:M])

            # Q0T = (I @ Wq0 + bq0) * SCALE, feature-major
            Q0T = sg.tile([128, DT, M], BF16)
            for m in range(DT):
                ps = ppA.tile([128, M], F32, tag="lin")
                for k in range(DT):
                    nc.tensor.matmul(ps, W["0q"][:, k, 128 * m:128 * (m + 1)],
                                     IT[:, k, :], start=(k == 0), stop=(k == DT - 1))
                nc.scalar.activation(Q0T[:, m, :], ps, AF.Identity,
                                     bias=bq0s_pp[:, m:m + 1], scale=float(SCALE))

            # Q0blk [128, HP, 2M] block-diag (head pair) for scores0
            Q0blk = sg.tile([128, HP, 2 * M], BF16)
            zero_bf(Q0blk, HP * 2 * M)
            for hp in range(HP):
                nc.vector.tensor_copy(Q0blk[0:64, hp, 0:M], Q0T[0:64, hp, :])
                nc.vector.tensor_copy(Q0blk[64:128, hp, M:2 * M], Q0T[64:128, hp, :])

            # Q0res_rep [128, D] bf16: 8x-replicated (I@Wq0 + bq0 + bv0), token-major
            r0 = sm.tile([1, D], F32, tag="bcrow")
            nc.sync.dma_start(out=r0, in_=p_d["0bqkv"][0][None, :])
            r1 = sm.tile([1, D], F32, tag="bcrow")
            nc.sync.dma_start(out=r1, in_=p_d["0bqkv"][2][None, :])
            nc.vector.tensor_add(r0, r0, r1)
            bqv0_bc = sg.tile([128, D], F32)
            nc.gpsimd.partition_broadcast(bqv0_bc, r0)
            ITrep = sg.tile([128, DT, 128], BF16)
            for k in range(DT):
                nc.vector.tensor_copy(ITrep[:, k, :],
                                      _ap(IT[:, k, :], [[0, 8], [1, M]]))
            psq = ppA.tile([128, D], F32, tag="lin")
            for k in range(DT):
                nc.tensor.matmul(psq, ITrep[:, k, :], W["0q"][:, k, :],
                                 start=(k == 0), stop=(k == DT - 1))
            Q0res_rep = sg.tile([128, D], BF16)
            nc.vector.tensor_add(Q0res_rep, psq, bqv0_bc)
            # isab2 variant adds the folded b1[1]@W0v row (A rows sum to 1)
            Q0res_rep2 = sg.tile([128, D], BF16)
            nc.vector.tensor_add(Q0res_rep2, Q0res_rep, bcast_row(r_b1W0v, "b1W0v"))

            # persistent zero-padded block-diag A^T stores (off-diag stays 0),
            # double-buffered by island/group parity so consecutive islands
            # don't serialize on WAR deps (zero pads persist per buffer)
            AQ0p, AQ1p, A1sbp, K1Tep, K1Top = [], [], [], [], []
            for pb in range(2):
                aq0 = sg.tile([128, 4, HP, 2, 4, M], BF16, name=f"AQ0_{pb}")
                zero_bf(aq0, 4 * HP * 2 * 4 * M)
                AQ0p.append(aq0)
                aq1 = sg.tile([128, H, 2, 4, S], BF16, name=f"AQ1_{pb}")
                zero_bf(aq1, H * 2 * 4 * S)
                AQ1p.append(aq1)
                a1 = sg.tile([128, H, 2, M], BF16, name=f"A1sb_{pb}")
                zero_bf(a1, H * 2 * M)
                A1sbp.append(a1)
                # K1T variants with one head-parity's rows zeroed (so scores1
                # contracts K=128 from row 0, avoiding the (64,96) PE quadrant)
                ke = sg.tile([128, DT, 256], BF16, name=f"K1Te_{pb}")
                zero_bf(ke, DT * 256)
                K1Tep.append(ke)
                ko = sg.tile([128, DT, 256], BF16, name=f"K1To_{pb}")
                zero_bf(ko, DT * 256)
                K1Top.append(ko)

            # ============ helpers ============
            def linear_fm(Wsb, inT, toks, bias_pp, tag):
                outT = wk.tile([128, DT, 512], BF16, tag=tag, bufs=2, name=f"fm_{tag}")
                for m in range(DT):
                    ps = ppA.tile([128, toks], F32, tag="lin")
                    for k in range(DT):
                        nc.tensor.matmul(ps, Wsb[:, k, 128 * m:128 * (m + 1)],
                                         inT[:, k, :toks], start=(k == 0), stop=(k == DT - 1))
                    if bias_pp is not None:
                        nc.scalar.activation(outT[:, m, :toks], ps, AF.Identity,
                                             bias=bias_pp[:, m:m + 1])
                    else:
                        nc.scalar.copy(outT[:, m, :toks], ps)
                return outT

            def ln_z(x_sb, ztag):
                """z = (x - mu) * rstd, bf16. rstd via Ln+Exp (one ACT table)."""
                st = sm.tile([128, 6], F32, tag="lnst")
                nc.vector.bn_stats(st, x_sb)
                mv = sm.tile([128, 2], F32, tag="lnmv")
                nc.vector.bn_aggr(mv, st)
                lnv = sm.tile([128, 1], F32, tag=f"{ztag}l")
                nc.scalar.activation(lnv, mv[:, 1:2], AF.Ln, bias=eps_t[:, 0:1])
                rstd = sm.tile([128, 1], F32, tag=f"{ztag}r")
                nc.scalar.activation(rstd, lnv, AF.Exp, scale=-0.5)
                z = sm.tile([128, D], BF16, tag=ztag, bufs=3, name=ztag)
                nc.vector.tensor_scalar(z, x_sb, mv[:, 0:1], rstd[:, 0:1],
                                        op0=ALU.subtract, op1=ALU.mult)
                return z

            def transpose_fm(isl, dst_ap):
                """PE-transpose island [128 toks, 512] bf16 -> feature-major via
                one 1-bank PSUM tile and a single merged copy."""
                ps = ppB.tile([128, DT, 128], BF16, tag="tp")
                for m in range(DT):
                    nc.tensor.transpose(ps[:, m, :], isl[:, 128 * m:128 * (m + 1)], id_bf)
                nc.scalar.copy(dst_ap, ps)

            # ============ one ISAB for one group of 16 batches ============
            def dummy_out(g):
                osb = sm.tile([G, D], F32, tag="osb", bufs=2)
                nc.vector.memset(osb, 0.0)
                nc.sync.dma_start(out=out_d[G * g:G * (g + 1), :], in_=osb)

            def isab(inT, g, last):
                gi = 2 * g + (1 if last else 0)   # group-isab index
                AQ0 = AQ0p[gi % 2]
                K1Te, K1To = K1Tep[gi % 2], K1Top[gi % 2]
                # weight/bias variants: isab2 consumes h1 = ln1-normalized only,
                # with mab1's g1/b1 folded into these
                Wk0 = W["0k"] if not last else W0k_2
                bk0v = bk0_pp if not last else bk0_2pp
                Wv0 = W["0v"] if not last else W0v_2
                Wq1 = W["1q"] if not last else W1q_2
                bq1v = bq1_pp if not last else bq1_2pp
                Q0res_v = Q0res_rep if not last else Q0res_rep2

                # ---- mab0: Hm = MAB(I, X) ----
                if STAGE < 2:
                    return None
                KT = linear_fm(Wk0, inT, 512, bk0v, tag="kt")
                V0t = wk.tile([128, 4, D], BF16, tag="v0t", bufs=2, name="v0t")
                for i in range(4):
                    ps = ppA.tile([128, D], F32, tag="lin")
                    for k in range(DT):
                        nc.tensor.matmul(ps, inT[:, k, 128 * i:128 * (i + 1)],
                                         Wv0[:, k, :], start=(k == 0), stop=(k == DT - 1))
                    nc.scalar.copy(V0t[:, i, :], ps)

                ps_s = ppA.tile([128, 512], F32, tag="lin")
                for hp in range(HP):
                    nc.tensor.matmul(ps_s[32 * hp:32 * (hp + 1), :], Q0blk[:, hp, :],
                                     KT[:, hp, :], start=True, stop=True,
                                     tile_position=(0, 32 * hp))
                E0 = sm.tile([128, 512], F32, tag="e0")
                nc.scalar.activation(E0, ps_s, AF.Exp)
                den = sm.tile([128, G], F32, tag="den0")
                nc.vector.tensor_reduce(den, E0.rearrange("p (b k) -> p b k", k=S),
                                        axis=AX.X, op=ALU.add)
                nc.vector.reciprocal(den, den)
                A0 = sm.tile([128, 512], BF16, tag="a0")
                nc.vector.tensor_tensor(
                    A0.rearrange("p (b k) -> p b k", k=S), E0.rearrange("p (b k) -> p b k", k=S),
                    _ap(den[:, :], [[1, G], [0, S]]), op=ALU.mult)
                # A^T for all 4 quads into the block-diag store
                A0v = A0.rearrange("p (j q k) -> p j q k", q=4, k=S)
                for hp in range(HP):
                    for gq in range(4):
                        nc.vector.transpose(
                            _ap(AQ0[32 * gq:32 * (gq + 1), 0, hp, 0, gq, :],
                                [[512, 4], [64, 2], [1, M]]),
                            A0v[32 * hp:32 * (hp + 1), :, gq, :])
                if STAGE < 3:
                    return None
                isl0 = []
                for jj in range(2):          # two islands of 128 tokens (8 batches)
                    ps_av = ppA.tile([128, D], F32, tag="lin")
                    # full-tile residual (I@Wq0+bq0+bv0) FIRST with start=True;
                    # AV matmuls then accumulate onto it (order-robust)
                    nc.tensor.matmul(ps_av, id_bf, Q0res_v,
                                     start=True, stop=False, skip_group_check=True)
                    for j2 in range(2):
                        j = 2 * jj + j2
                        for hp in range(HP):
                            for i in range(2):
                                h = 2 * hp + i
                                nc.tensor.matmul(
                                    ps_av[64 * j2:64 * j2 + 64, 64 * h:64 * (h + 1)],
                                    AQ0[:, j, hp, i, :, :].rearrange("p g q -> p (g q)"),
                                    V0t[:, j, 64 * h:64 * (h + 1)],
                                    start=False,
                                    stop=(j2 == 1 and hp == HP - 1 and i == 1),
                                    tile_position=(0, 64 * j2), skip_group_check=True)
                    O0 = sm.tile([128, D], BF16, tag="o0", bufs=3)
                    nc.scalar.copy(O0, ps_av)
                    z0 = ln_z(O0, "z0")
                    # L0 = g0*z0 + b0 (token-major residual for the fc block)
                    t0 = sm.tile([128, D], BF16, tag="lnt1", bufs=3)
                    nc.vector.tensor_tensor(t0, z0, ln_bc["0g0"], op=ALU.mult)
                    L0 = sm.tile([128, D], BF16, tag="ln0out", bufs=3)
                    nc.vector.tensor_tensor(L0, t0, ln_bc["0b0"], op=ALU.add)
                    OT = wk.tile([128, DT, 128], BF16, tag="ot", bufs=2)
                    transpose_fm(z0, OT[:, :, :])
                    psf = ppA.tile([128, D], F32, tag="lin")
                    nc.tensor.matmul(psf, ones128, bo_bc[0],
                                     start=True, stop=False, skip_group_check=True)
                    for k in range(DT):
                        nc.tensor.matmul(psf, OT[:, k, :], W["0o"][:, k, :],
                                         start=False, stop=(k == DT - 1),
                                         skip_group_check=True)
                    O2 = sm.tile([128, D], BF16, tag="o2", bufs=3)
                    nc.vector.scalar_tensor_tensor(O2, psf, 0.0, L0,
                                                   op0=ALU.max, op1=ALU.add)
                    isl0.append(ln_z(O2, "lnN"))
                HT = wk.tile([128, DT, 272], BF16, tag="ht", bufs=2)
                for t2 in range(2):
                    transpose_fm(isl0[t2], HT[:, :, 128 * t2:128 * t2 + 128])

                # ---- mab1: out = MAB(X, Hm) ----
                if STAGE < 4:
                    return None
                Q1Tb = wk.tile([128, DT, 512], BF16, tag="q1b", bufs=2, name="q1b")
                for m in range(DT):
                    ps = ppA.tile([128, 512], F32, tag="lin")
                    for k in range(DT):
                        nc.tensor.matmul(ps, Wq1[:, k, 128 * m:128 * (m + 1)],
                                         inT[:, k, :], start=(k == 0), stop=(k == DT - 1))
                    nc.scalar.activation(Q1Tb[:, m, :], ps, AF.Identity,
                                         bias=bq1v[:, m:m + 1])
                for m in range(DT):
                    ps = ppA.tile([128, 256], F32, tag="lin")
                    for k in range(DT):
                        nc.tensor.matmul(ps, W["1k"][:, k, 128 * m:128 * (m + 1)],
                                         HT[:, k, :256], start=(k == 0), stop=(k == DT - 1))
                    nc.scalar.activation(K1Te[0:64, m, :], ps[0:64, :], AF.Identity,
                                         bias=bk1s_pp[0:64, m:m + 1])
                    nc.scalar.activation(K1To[64:128, m, :], ps[64:128, :], AF.Identity,
                                         bias=bk1s_pp[64:128, m:m + 1])
                V1t = wk.tile([128, 2, D], BF16, tag="v1t", bufs=2, name="v1t")
                for i in range(2):
                    ps = ppA.tile([128, D], F32, tag="lin")
                    for k in range(DT):
                        nc.tensor.matmul(ps, HT[:, k, 128 * i:128 * (i + 1)],
                                         W["1v"][:, k, :], start=(k == 0),
                                         stop=(k == DT - 1))
                    nc.scalar.copy(V1t[:, i, :], ps)

                if STAGE < 5:
                    return None
                if not last:
                    h1T = wk.tile([128, DT, 512], BF16, tag="fmX", bufs=2, name="h1T")
                else:
                    macc = sm.tile([128, DT, G], F32, tag="macc")
                # ---- scores in 2 waves of 2 quads; one MM per (quad, head)
                # covering all 4 batches (3/4 garbage cols, never read) ----
                for w in range(2):
                  ps_w = ppB.tile([128, 2, H, 4, M], F32, tag="s1", bufs=1)
                  for j2 in range(2):
                    j = 2 * w + j2
                    for h in range(H):
                        hp, i = h // 2, h % 2
                        K1v = K1Te if i == 0 else K1To
                        nc.tensor.matmul(
                            ps_w[:, j2, h, :, :].rearrange("p b k -> p (b k)"),
                            Q1Tb[:, hp, 128 * j:128 * (j + 1)],
                            K1v[:, hp, M * 4 * j:M * 4 * (j + 1)],
                            start=True, stop=True)
                  E1w = sm.tile([128, 2, H, 4, M], F32, tag="e1")
                  nc.scalar.activation(E1w.rearrange("p a h b k -> p (a h b k)"),
                                       ps_w.rearrange("p a h b k -> p (a h b k)"),
                                       AF.Exp)
                  den1 = sm.tile([128, 2, H, 4], F32, tag="den1")
                  nc.vector.tensor_reduce(den1, E1w, axis=AX.X, op=ALU.add)
                  nc.vector.reciprocal(den1.rearrange("p a h b -> p (a h b)"),
                                       den1.rearrange("p a h b -> p (a h b)"))
                  for j2 in range(2):
                    j = 2 * w + j2
                    j8, half = j // 2, j % 2
                    A1sb = A1sbp[j % 2]
                    AQ1 = AQ1p[j % 2]
                    # normalize valid diagonal blocks into A1sb (pad stays zero)
                    for gq in range(4):
                        sl = slice(32 * gq, 32 * (gq + 1))
                        nc.vector.tensor_tensor(
                            A1sb[sl, :, gq % 2, :], E1w[sl, j2, :, gq, :],
                            _ap(den1[sl, j2, :, gq], [[4, H], [0, M]]), op=ALU.mult)
                    # A^T blocks: [32 q, (b2,k)] -> [32 (b2,k), q] at row 32*(2*half+gq//2)
                    for gq in range(4):
                        prow = 32 * (2 * half + gq // 2)
                        nc.vector.transpose(
                            _ap(AQ1[prow:prow + 32, 0, half, gq, :],
                                [[2 * 4 * S, H], [1, S]]),
                            A1sb[32 * gq:32 * (gq + 1), :, :, :])
                    if STAGE == 43:
                        continue
                    ps_av1 = ppA.tile([128, D], F32, tag="lin")
                    # bv1 broadcast FIRST covering the full tile (A rows sum to 1,
                    # so A@(V+bv1) == A@V + bv1), then residual Q1^T transposes and
                    # AV matmuls accumulate onto it
                    nc.tensor.matmul(ps_av1, ones128, bv1_bc,
                                     start=True, stop=False, skip_group_check=True)
                    for m in range(DT):
                        nc.tensor.matmul(ps_av1[:, 128 * m:128 * (m + 1)],
                                         Q1Tb[:, m, 128 * j:128 * (j + 1)], id_bf,
                                         start=False, stop=False,
                                         skip_group_check=True)
                    for h in range(H):
                        nc.tensor.matmul(ps_av1[:, 64 * h:64 * (h + 1)],
                                         AQ1[:, h, half, :, :].rearrange("p g q -> p (g q)"),
                                         V1t[:, j8, 64 * h:64 * (h + 1)],
                                         start=False, stop=(h == H - 1),
                                         skip_group_check=True)
                    if STAGE == 44:
                        continue
                    O1 = sm.tile([128, D], BF16, tag="o0", bufs=3)
                    nc.scalar.copy(O1, ps_av1)
                    z1 = ln_z(O1, "z0")
                    t1 = sm.tile([128, D], BF16, tag="lnt1", bufs=3)
                    nc.vector.tensor_tensor(t1, z1, ln_bc["1g0"], op=ALU.mult)
                    L1 = sm.tile([128, D], BF16, tag="ln0out", bufs=3)
                    nc.vector.tensor_tensor(L1, t1, ln_bc["1b0"], op=ALU.add)
                    OT1 = wk.tile([128, DT, 128], BF16, tag="ot", bufs=2)
                    transpose_fm(z1, OT1[:, :, :])
                    psf = ppA.tile([128, D], F32, tag="lin")
                    nc.tensor.matmul(psf, ones128, bo_bc[1],
                                     start=True, stop=False, skip_group_check=True)
                    for k in range(DT):
                        nc.tensor.matmul(psf, OT1[:, k, :], W["1o"][:, k, :],
                                         start=False, stop=(k == DT - 1),
                                         skip_group_check=True)
                    O2 = sm.tile([128, D], BF16, tag="o2", bufs=3)
                    nc.vector.scalar_tensor_tensor(O2, psf, 0.0, L1,
                                                   op0=ALU.max, op1=ALU.add)
                    OUTj = ln_z(O2, "lnN")
                    if not last:
                        transpose_fm(OUTj, h1T[:, :, 128 * j:128 * j + 128])
                    else:
                        ps = ppB.tile([128, DT, 128], BF16, tag="tp")
                        for m in range(DT):
                            nc.tensor.transpose(ps[:, m, :], OUTj[:, 128 * m:128 * (m + 1)], id_bf)
                        nc.vector.tensor_reduce(
                            macc[:, :, 4 * j:4 * (j + 1)],
                            ps.rearrange("p m (b s) -> p m b s", s=S),
                            axis=AX.X, op=ALU.add)
                if STAGE < 46:
                    return None
                if not last:
                    return h1T
                # pooled = g1/S * sum + b1, applied feature-major, then transpose out
                macc2 = sm.tile([128, DT, G], F32, tag="macc2")
                for m in range(DT):
                    nc.scalar.activation(macc2[:, m, :], macc[:, m, :], AF.Identity,
                                         bias=b1_32[:, m:m + 1], scale=g1s_pp[:, m:m + 1])
                osb = sm.tile([G, D], F32, tag="osb", bufs=2)
                for m in range(DT):
                    ps = ppC.tile([128, 128], F32, tag="mp")
                    nc.tensor.transpose(ps[0:G, :], macc2[:, m, :], id_f32)
                    nc.scalar.copy(osb[:, 128 * m:128 * (m + 1)], ps[0:G, :])
                nc.sync.dma_start(out=out_d[G * g:G * (g + 1), :], in_=osb)
                return None

            # ============ main loop ============
            x_flat = x_d.rearrange("b s d -> (b s) d")
            for g in range(ngroups):
                Xb = wk.tile([128, 4, D], F32, tag="xb", bufs=2)
                for i in range(4):
                    nc.sync.dma_start(
                        out=Xb[:, i, :],
                        in_=x_flat[512 * g + 128 * i: 512 * g + 128 * (i + 1), :])
                Xbb = wk.tile([128, 4, D], BF16, tag="xbb", bufs=2)
                nc.vector.tensor_copy(Xbb, Xb)
                XT = wk.tile([128, DT, 512], BF16, tag="fmX", bufs=2, name="XT")
                for i in range(4):
                    ps = ppB.tile([128, DT, 128], BF16, tag="tp")
                    for m in range(DT):
                        nc.tensor.transpose(ps[:, m, :], Xbb[:, i, 128 * m:128 * (m + 1)], id_bf)
                    nc.scalar.copy(XT[:, :, 128 * i:128 * (i + 1)], ps)
                h1T = isab(XT, g, last=False)
                if h1T is None or STAGE < 6:
                    dummy_out(g)
                    continue
                isab(h1T, g, last=True)

    _orig = bacc.get_activation_tables
    bacc.get_activation_tables = _patched_act_tables(_orig)
    try:
        nc.finalize()
    finally:
        bacc.get_activation_tables = _orig
    return nc


_CACHE = {}


def _get_nc(nb):
    if nb not in _CACHE:
        _CACHE[nb] = build(nb)
    return _CACHE[nb]


def kernel(**inputs):
    from concourse.bass_utils import run_bass_kernel_spmd

    x = np.ascontiguousarray(inputs["x"], dtype=np.float32)
    nbatch = x.shape[0]
    per = nbatch // NCORES
    nc = _get_nc(per)
    shared = {k: np.ascontiguousarray(np.asarray(v), dtype=np.float32)
              for k, v in inputs.items() if k != "x"}
    in_maps = [dict(shared, x=x[c * per:(c + 1) * per]) for c in range(NCORES)]
    res = run_bass_kernel_spmd(nc, in_maps, core_ids=list(range(NCORES)))
    return np.concatenate([r["out"] for r in res.results], axis=0)
